# revision 14
# baseline (speedup 1.0000x reference)
"""Trainium2 Bass kernel for the news-attention module (v5).

Math (exact, unchanged):
  relu(hc + hh) = max(hc, -hh) + hh
  logits a[n,h] = w2 . max(hc[n], -hh[h]) + sh[h],  sh = w2 . hh
  softmax(a + maskbias) = exp(w2.max-part) * m[h] / sum,
      m[h] = mask[h] * exp(sh[h])
  ur[n] = (1/ssum) sum_h exnorm[n,h] * (m[h] * histf[h])

v5 = v4 pipeline + masked-history compaction: rows with mask=0
contribute exactly 0 (m[h]=0), so the host gathers only kept rows and
pads each batch to HP (= even ceil of the max kept count, <= 50); pad
rows get mask 0 and behave identically to masked rows. The entire
history pipeline (hh GEMM, max pass, matvec, attention K) runs at HP
instead of 50. The module is compiled per HP on first use.

Sharding: data-parallel over batch, 8 batches per core on 8 cores.
"""

import sys

for _p in ("/opt/trn_rl_repo",):
    if _p not in sys.path:
        sys.path.insert(0, _p)

import numpy as np
import ml_dtypes

import concourse.bass as bass
import concourse.bacc as bacc
import concourse.tile as tile
from concourse import mybir
from concourse import bass_utils
from concourse.masks import make_identity

DT = mybir.dt.float32
BF = mybir.dt.bfloat16
I32 = mybir.dt.int32
AF = mybir.ActivationFunctionType
ALU = mybir.AluOpType

NCORES = 8
B = 64
BC = B // NCORES  # 8 batches per core
H = 50
N = 50
D = 400
P = 100
A = 200
F = D + P       # 500
J = 52
A0 = 128        # a-chunk split: [0:128), [128:200)
A1 = A - A0     # 72
NT = 7          # matvec n-tiles per batch (8 n-locals each)
NG = 4          # batch groups (2 batches each)
GB = BC // NG   # batches per group = 2
NSG = NT * GB   # 14 psum slots per group


def _bc_(v, pos, n):
    """Insert a zero-stride (broadcast) dim of length n at position pos."""
    ap = [list(x) for x in v.ap]
    ap.insert(pos, [0, n])
    return bass.AP(tensor=v.tensor, offset=v.offset, ap=ap)


def _ap(v, offset_delta, ap_list):
    return bass.AP(tensor=v.tensor, offset=v.offset + offset_delta, ap=ap_list)


def _body(nc, hist_in, cand_in, maskT_in, pos_in, pos_emb16, pos_emb32,
          posT16, w1t, b1_16, w2_16, ur_out, cand_out, tc, HP):
    import contextlib

    HW = 32 if HP <= 32 else 64          # psum h-padding per n-local
    NWIN = (8 * HW) // 128               # transpose windows per slot row
    PARS = 128 // HW                     # n-locals per window

    ctx = contextlib.ExitStack()
    with ctx:
        consts = ctx.enter_context(tc.tile_pool(name="consts", bufs=1))
        ps = ctx.enter_context(tc.tile_pool(name="ps", bufs=3, space="PSUM"))
        psmv = ctx.enter_context(tc.tile_pool(name="psmv", bufs=2, space="PSUM"))
        psat = ctx.enter_context(tc.tile_pool(name="psat", bufs=2, space="PSUM"))
        hidp = ctx.enter_context(tc.tile_pool(name="hid", bufs=3))
        exp_ = ctx.enter_context(tc.tile_pool(name="exp", bufs=2))

        # ---------------- constants ----------------
        ident16 = consts.tile([128, 128], BF)
        make_identity(nc, ident16)
        ident32 = consts.tile([128, 128], DT)
        make_identity(nc, ident32)

        # W1T[f, a] in 10 f-chunks of 100 (host provides W1 transposed, bf16)
        w1T = consts.tile([100, 10, A], BF)
        nc.sync.dma_start(out=w1T,
                          in_=w1t.ap().rearrange("(k p) a -> p k a", p=100))

        pos_emb_s = consts.tile([J, P], BF)
        nc.gpsimd.dma_start(out=pos_emb_s, in_=pos_emb16.ap())
        posT = consts.tile([P, J], BF)
        nc.gpsimd.dma_start(out=posT, in_=posT16.ap())
        maskT_s = consts.tile([HP, BC], DT)
        nc.gpsimd.dma_start(out=maskT_s, in_=maskT_in.ap())
        b1row = consts.tile([1, A], BF)
        nc.gpsimd.dma_start(out=b1row, in_=_ap(b1_16.ap(), 0, [[0, 1], [1, A]]))
        one11 = consts.tile([1, 1], BF)
        nc.vector.memset(one11, 1.0)

        # w2 chunks as columns + shifted zero-window tensors for slot matvec
        w2colA = consts.tile([A0, 1], BF)
        nc.gpsimd.dma_start(out=w2colA, in_=_ap(w2_16.ap(), 0, [[1, A0], [1, 1]]))
        w2colB = consts.tile([A1, 1], BF)
        nc.gpsimd.dma_start(out=w2colB, in_=_ap(w2_16.ap(), A0, [[1, A1], [1, 1]]))
        ZA = consts.tile([A0, 2 * NSG - 1], BF)
        ZB = consts.tile([A1, 2 * NSG - 1], BF)
        nc.vector.memset(ZA, 0.0)
        nc.vector.memset(ZB, 0.0)
        nc.vector.tensor_copy(out=ZA[:, NSG - 1:NSG], in_=w2colA)
        nc.vector.tensor_copy(out=ZB[:, NSG - 1:NSG], in_=w2colB)

        # E[j, a] = pos_emb @ Wh2.T  (Wh2 = W1[:, 900:1000] rows of w1t)
        E_s = consts.tile([J, A], BF)
        psE = ps.tile([J, A], DT, tag="ps")
        nc.tensor.matmul(psE, lhsT=posT[:, :], rhs=w1T[:, 9, :],
                         start=True, stop=True)
        nc.scalar.copy(out=E_s, in_=psE)

        # c0[a] = Wc2 @ pos_emb[1] + b1 as per-partition bias columns
        c0colA = consts.tile([A0, 1], DT)
        c0colB = consts.tile([A1, 1], DT)
        for c0c, asl in ((c0colA, slice(0, A0)), (c0colB, slice(A0, A))):
            psc = ps.tile([c0c.shape[0], 1], DT, tag="ps")
            nc.tensor.matmul(psc, lhsT=w1T[:, 4, asl], rhs=posT[:, 1:2],
                             start=True, stop=False)
            nc.tensor.matmul(psc, lhsT=b1row[:, asl], rhs=one11[:, :],
                             start=False, stop=True)
            nc.scalar.copy(out=c0c, in_=psc)

        # one-hot of positions, transposed: onehot[j, b*HP+h] = (pos[b,h]==j)
        pos52 = consts.tile([J, BC * HP], I32)
        nc.gpsimd.dma_start(out=pos52, in_=_bc_(pos_in.ap(), 0, J))
        iot = consts.tile([J, BC * HP], I32)
        nc.gpsimd.iota(iot, pattern=[[0, BC * HP]], base=0, channel_multiplier=1)
        onehot = consts.tile([J, BC * HP], BF)
        nc.vector.tensor_tensor(out=onehot, in0=iot, in1=pos52, op=ALU.is_equal)

        # ---------------- full-input loads --------------------------------
        cand_all = consts.tile([100, 4, D], DT)   # [2x50 rows, batch-pair, feat]
        hist_all = consts.tile([2 * HP, 4, D], DT)
        for hf in range(2):
            src_c = _ap(cand_in.ap(), hf * N * D,
                        [[D, 50], [2 * N * D, 4], [1, D]])
            src_h = _ap(hist_in.ap(), hf * HP * D,
                        [[D, HP], [2 * HP * D, 4], [1, D]])
            nc.sync.dma_start(out=cand_all[hf * 50:(hf + 1) * 50, :, :],
                              in_=src_c)
            nc.sync.dma_start(out=hist_all[hf * HP:(hf + 1) * HP, :, :],
                              in_=src_h)

        # candidate passthrough: two strided DMAs + pos_emb[1] broadcast
        for hf in range(2):
            dst = _ap(cand_out.ap(), hf * N * F,
                      [[F, 50], [2 * N * F, 4], [1, D]])
            nc.sync.dma_start(out=dst, in_=cand_all[hf * 50:(hf + 1) * 50, :, :])
        nc.gpsimd.dma_start(
            out=cand_out.ap()[:, :, D:F],
            in_=_bc_(_bc_(pos_emb32.ap()[1:2, :], 0, N), 0, BC))

        # hist rows re-staged [h, b, f] for the attention V matrix
        staging = consts.tile([HP, BC, D], DT)
        src_hf = _ap(hist_in.ap(), 0, [[D, HP], [HP * D, BC], [1, D]])
        nc.sync.dma_start(out=staging, in_=src_hf)

        # ---------------- persistent SBUF tensors -------------------------
        candT = consts.tile([100, 4, BC * N], BF)  # [feat-chunk, k, (b,n)]
        histT = consts.tile([100, 4, BC * HP], BF)
        hcT2A = consts.tile([A0, BC * N, 2], BF)
        hcT2B = consts.tile([A1, BC * N, 2], BF)
        neghhA = consts.tile([A0, BC * HP], BF)
        neghhB = consts.tile([A1, BC * HP], BF)
        mw = consts.tile([HP, BC], DT)
        histf16 = consts.tile([HP, BC, F + 1], BF)
        alphaT = consts.tile([HP, BC, 64], BF)
        urs = consts.tile([N, BC, F], DT)
        rs_all = consts.tile([N, BC], DT)

        # ---------------- transposes (all groups) + wide GEMMs ------------
        for g in range(4):
            ptc = ps.tile([100, 4, 100], DT, tag="ps")
            pth = ps.tile([100, 4, 2 * HP], DT, tag="ps")
            for k in range(4):
                nc.tensor.transpose(
                    ptc[:, k, :], cand_all[:, g, k * 100:(k + 1) * 100],
                    ident32[:100, :100])
                nc.tensor.transpose(
                    pth[:, k, :], hist_all[:, g, k * 100:(k + 1) * 100],
                    ident32[:2 * HP, :2 * HP])
            nc.scalar.copy(out=candT[:, :, g * 100:(g + 1) * 100], in_=ptc)
            nc.scalar.copy(out=histT[:, :, g * 2 * HP:(g + 1) * 2 * HP],
                           in_=pth)

        for asl, hcT2, neghh, c0c in (
                (slice(0, A0), hcT2A, neghhA, c0colA),
                (slice(A0, A), hcT2B, neghhB, c0colB)):
            m = asl.stop - asl.start
            pg = ps.tile([m, BC * N], DT, tag="ps")
            for k in range(4):
                nc.tensor.matmul(pg, lhsT=w1T[:, k, asl], rhs=candT[:, k, :],
                                 start=(k == 0), stop=(k == 3))
            nc.scalar.activation(out=hcT2, in_=_bc_(pg[:, :], 2, 2),
                                 func=AF.Identity, bias=c0c, scale=1.0)
            ph = ps.tile([m, BC * HP], DT, tag="ps")
            for k in range(4):
                nc.tensor.matmul(ph, lhsT=w1T[:, 5 + k, asl],
                                 rhs=histT[:, k, :],
                                 start=(k == 0), stop=False)
            nc.tensor.matmul(ph, lhsT=E_s[:, asl], rhs=onehot[:, :],
                             start=False, stop=True)
            nc.scalar.activation(out=neghh, in_=ph, func=AF.Copy, scale=-1.0)

        # ---------------- per-group prep: shT + mw + histf ----------------
        def head(g):
            psh = ps.tile([HP, GB], DT, tag="ps")
            for bl in range(GB):
                b = g * GB + bl
                hsl = slice(b * HP, (b + 1) * HP)
                nc.tensor.matmul(psh[:, bl:bl + 1], lhsT=neghhA[:, hsl],
                                 rhs=w2colA, start=True, stop=False)
                nc.tensor.matmul(psh[:, bl:bl + 1], lhsT=neghhB[:, hsl],
                                 rhs=w2colB, start=False, stop=True)
            shE = exp_.tile([HP, GB], DT, tag="she")
            nc.scalar.activation(out=shE, in_=psh, func=AF.Exp, scale=-1.0)
            bsl = slice(g * GB, (g + 1) * GB)
            nc.vector.tensor_tensor(out=mw[:, bsl], in0=shE,
                                    in1=maskT_s[:, bsl], op=ALU.mult)

            # histf (V matrix) with m-scale + m column (ssum source);
            # pos-emb part straight from psum.
            for bl in range(GB):
                b = g * GB + bl
                nc.scalar.activation(out=histf16[:, b, 0:D],
                                     in_=staging[:, b, :],
                                     func=AF.Copy, scale=mw[:, b:b + 1])
                ppg = ps.tile([HP, P], DT, tag="ps")
                nc.tensor.matmul(ppg, lhsT=onehot[:, b * HP:(b + 1) * HP],
                                 rhs=pos_emb_s[:, :], start=True, stop=True)
                nc.scalar.activation(out=histf16[:, b, D:F], in_=ppg,
                                     func=AF.Copy, scale=mw[:, b:b + 1])
                nc.scalar.copy(out=histf16[:, b, F:F + 1], in_=mw[:, b:b + 1])

        # ---------------- per-group matvec (max pass + slot matmuls) ------
        def matvec(g):
            pmv = psmv.tile([NSG, 8, HW], DT, tag="pmv")
            first = True
            for bl in range(GB):
                b = g * GB + bl
                nsl = slice(b * N, (b + 1) * N)
                hids = []
                for hcT2, neghh, aw in ((hcT2A, neghhA, A0),
                                        (hcT2B, neghhB, A1)):
                    hid = hidp.tile([aw, N * HP], BF, tag=f"hid{aw}")
                    v = hcT2[:, nsl, :]
                    hcb = _bc_(v, 2, HP // 2)           # [aw, 50, HP/2, 2]
                    w = neghh[:, b * HP:(b + 1) * HP]
                    hhb = _ap(w, 0, [list(w.ap[0]), [0, N], [2, HP // 2],
                                     [1, 2]])
                    hidv = _ap(hid, 0, [list(hid.ap[0]), [HP, N],
                                        [2, HP // 2], [1, 2]])
                    nc.vector.tensor_tensor(out=hidv, in0=hcb, in1=hhb,
                                            op=ALU.max)
                    hids.append(hid)
                for t in range(NT):
                    s = t * GB + bl
                    nn = 8 if t < 6 else 2   # n-cols in this tile
                    rsl = slice(t * 8 * HP, (t * 8 + nn) * HP)
                    dst = _ap(pmv, 0, [list(pmv.ap[0]), [HW, nn], [1, HP]])
                    for Z, hv in ((ZA, hids[0]), (ZB, hids[1])):
                        last = (bl == GB - 1 and t == NT - 1 and Z is ZB)
                        nc.tensor.matmul(
                            dst, lhsT=Z[:, NSG - 1 - s:2 * NSG - 1 - s],
                            rhs=hv[:, rsl], start=first, stop=last)
                        first = False
            return pmv

        # ---------------- per-group tail: exp -> alpha -> attention -------
        def tail(g, pmv):
            ex = exp_.tile([NSG, 8, HW], BF, tag="ex")
            nc.scalar.activation(out=ex[:, :, 0:HP], in_=pmv[:, :, 0:HP],
                                 func=AF.Exp)
            ptr = ps.tile([128, NWIN, NSG], BF, tag="ps")
            for w in range(NWIN):
                nc.tensor.transpose(
                    ptr[:, w, :],
                    _ap(ex, w * 128, [list(ex.ap[0]), [1, 128]]),
                    ident16[:NSG, :NSG])
            exT = exp_.tile([128, NWIN, NSG], BF, tag="exT")
            nc.scalar.copy(out=exT, in_=ptr)
            # alpha assembly on Pool: alphaT[h, b, n], n = 8t + PARS*w + par
            # (one copy per par covers all windows w)
            for par in range(PARS):
                src = exT[par * HW:par * HW + HP, :, :]
                nc.gpsimd.tensor_copy(
                    out=_ap(alphaT, g * GB * 64 + par,
                            [list(alphaT.ap[0]), [PARS, NWIN], [8, NT],
                             [64, GB]]),
                    in_=_ap(src, 0, [list(src.ap[0]), [NSG, NWIN], [GB, NT],
                                     [1, GB]]))
            # attention: ur = alpha @ histf / ssum
            for bl in range(GB):
                b = g * GB + bl
                pur = psat.tile([N, F + 1], DT, tag="pur")
                nc.tensor.matmul(pur, lhsT=alphaT[:, b, 0:N],
                                 rhs=histf16[:, b, :], start=True, stop=True)
                nc.vector.reciprocal(rs_all[:, b:b + 1], pur[:, F:F + 1])
                nc.scalar.activation(out=urs[:, b, :], in_=pur[:, 0:F],
                                     func=AF.Copy, scale=rs_all[:, b:b + 1])
            dst_ur = _ap(ur_out.ap(), g * GB * N * F,
                         [[F, N], [N * F, GB], [1, F]])
            nc.sync.dma_start(out=dst_ur, in_=urs[:, g * GB:(g + 1) * GB, :])

        # ---------------- software pipeline ------------------------------
        head(0)
        head(1)
        pmv0 = matvec(0)
        head(2)
        pmv1 = matvec(1)
        tail(0, pmv0)
        head(3)
        pmv2 = matvec(2)
        tail(1, pmv1)
        pmv3 = matvec(3)
        tail(2, pmv2)
        tail(3, pmv3)


def build(debug=False, reps=1, loop=1, HP=H):
    nc = bacc.Bacc("TRN2", target_bir_lowering=False, debug=debug)
    hist_in = nc.dram_tensor("hist_in", [BC, HP, D], DT, kind="ExternalInput")
    cand_in = nc.dram_tensor("cand_in", [BC, N, D], DT, kind="ExternalInput")
    maskT_in = nc.dram_tensor("maskT_in", [HP, BC], DT, kind="ExternalInput")
    pos_in = nc.dram_tensor("pos_in", [BC, HP], I32, kind="ExternalInput")
    pos_emb16 = nc.dram_tensor("pos_emb16", [J, P], BF, kind="ExternalInput")
    pos_emb32 = nc.dram_tensor("pos_emb32", [J, P], DT, kind="ExternalInput")
    posT16 = nc.dram_tensor("posT16", [P, J], BF, kind="ExternalInput")
    w1t = nc.dram_tensor("w1t", [2 * F, A], BF, kind="ExternalInput")
    b1_16 = nc.dram_tensor("b1_16", [A], BF, kind="ExternalInput")
    w2_16 = nc.dram_tensor("w2_16", [A], BF, kind="ExternalInput")
    ur_out = nc.dram_tensor("ur_out", [BC, N, F], DT, kind="ExternalOutput")
    cand_out = nc.dram_tensor("cand_out", [BC, N, F], DT, kind="ExternalOutput")

    with tile.TileContext(nc) as tc:
        if loop > 1:
            with tc.For_i(0, loop):
                _body(nc, hist_in, cand_in, maskT_in, pos_in, pos_emb16,
                      pos_emb32, posT16, w1t, b1_16, w2_16, ur_out,
                      cand_out, tc, HP)
        else:
            for _ in range(reps):
                _body(nc, hist_in, cand_in, maskT_in, pos_in, pos_emb16,
                      pos_emb32, posT16, w1t, b1_16, w2_16, ur_out,
                      cand_out, tc, HP)
    nc.compile()
    return nc


_NCS = {}


def _get_nc(HP):
    if HP not in _NCS:
        _NCS[HP] = build(debug=False, HP=HP)
    return _NCS[HP]


def _bf(x):
    return np.ascontiguousarray(np.asarray(x, np.float32).astype(
        ml_dtypes.bfloat16))


def choose_hp(user_history_mask):
    k = int(np.asarray(user_history_mask).astype(bool).sum(axis=1).max())
    hp = max(2, k + (k % 2))
    return min(hp, H)


def compact(history_repr, user_history_mask, user_history_position, HP):
    """Gather kept (mask=1) history rows, pad to HP; pad rows get mask 0."""
    hist = np.asarray(history_repr, np.float32)
    mask = np.asarray(user_history_mask).astype(bool)
    pos = np.asarray(user_history_position).astype(np.int32)
    Bn = hist.shape[0]
    hist_g = np.zeros((Bn, HP, hist.shape[2]), np.float32)
    pos_g = np.zeros((Bn, HP), np.int32)
    mask_g = np.zeros((Bn, HP), np.float32)
    for b in range(Bn):
        idx = np.flatnonzero(mask[b])[:HP]
        k = len(idx)
        hist_g[b, :k] = hist[b, idx]
        pos_g[b, :k] = pos[b, idx]
        mask_g[b, :k] = 1.0
    return hist_g, pos_g, mask_g


def make_in_maps(history_repr, candidate_repr, user_history_mask,
                 user_history_position, pos_emb, W1, b1, w2, HP=None):
    if HP is None:
        HP = choose_hp(user_history_mask)
    hist_g, pos_g, mask_g = compact(history_repr, user_history_mask,
                                    user_history_position, HP)
    cand = np.ascontiguousarray(np.asarray(candidate_repr, np.float32))
    pe32 = np.ascontiguousarray(np.asarray(pos_emb, np.float32))
    pe16 = _bf(pe32)
    peT16 = _bf(pe32.T)
    w1t16 = _bf(np.asarray(W1, np.float32).T)
    b1_16 = _bf(b1)
    w2_16 = _bf(w2)
    in_maps = []
    for c in range(NCORES):
        sl = slice(c * BC, (c + 1) * BC)
        in_maps.append({
            "hist_in": np.ascontiguousarray(hist_g[sl]),
            "cand_in": cand[sl],
            "maskT_in": np.ascontiguousarray(mask_g[sl].T),
            "pos_in": np.ascontiguousarray(pos_g[sl]),
            "pos_emb16": pe16, "pos_emb32": pe32, "posT16": peT16,
            "w1t": w1t16, "b1_16": b1_16, "w2_16": w2_16,
        })
    return in_maps


def kernel(history_repr, candidate_repr, user_history_mask,
           user_history_position, pos_emb, W1, b1, w2, b2=None, **_ignored):
    # b2 shifts every logit equally -> cancels in softmax; unused.
    HP = choose_hp(user_history_mask)
    nc = _get_nc(HP)
    in_maps = make_in_maps(history_repr, candidate_repr, user_history_mask,
                           user_history_position, pos_emb, W1, b1, w2, HP=HP)
    res = bass_utils.run_bass_kernel_spmd(nc, in_maps, list(range(NCORES)))
    ur = np.concatenate([res.results[c]["ur_out"] for c in range(NCORES)], 0)
    cand = np.concatenate([res.results[c]["cand_out"] for c in range(NCORES)], 0)
    return ur, cand


# revision 16
# speedup vs baseline: 1.2103x; 1.2103x over previous
"""Trainium2 Bass kernel for the news-attention module (v5).

Math (exact, unchanged):
  relu(hc + hh) = max(hc, -hh) + hh
  logits a[n,h] = w2 . max(hc[n], -hh[h]) + sh[h],  sh = w2 . hh
  softmax(a + maskbias) = exp(w2.max-part) * m[h] / sum,
      m[h] = mask[h] * exp(sh[h])
  ur[n] = (1/ssum) sum_h exnorm[n,h] * (m[h] * histf[h])

v5 = v4 pipeline + masked-history compaction: rows with mask=0
contribute exactly 0 (m[h]=0), so the host gathers only kept rows and
pads each batch to HP (= even ceil of the max kept count, <= 50); pad
rows get mask 0 and behave identically to masked rows. The entire
history pipeline (hh GEMM, max pass, matvec, attention K) runs at HP
instead of 50. The module is compiled per HP on first use.

Sharding: data-parallel over batch, 8 batches per core on 8 cores.
"""

import sys

for _p in ("/opt/trn_rl_repo",):
    if _p not in sys.path:
        sys.path.insert(0, _p)

import numpy as np
import ml_dtypes

import concourse.bass as bass
import concourse.bacc as bacc
import concourse.tile as tile
from concourse import mybir
from concourse import bass_utils
from concourse.masks import make_identity

DT = mybir.dt.float32
BF = mybir.dt.bfloat16
I32 = mybir.dt.int32
AF = mybir.ActivationFunctionType
ALU = mybir.AluOpType

NCORES = 8
B = 64
BC = B // NCORES  # 8 batches per core
H = 50
N = 50
D = 400
P = 100
A = 200
F = D + P       # 500
J = 52
A0 = 128        # a-chunk split: [0:128), [128:200)
A1 = A - A0     # 72
NT = 7          # matvec n-tiles per batch (8 n-locals each)
NG = 4          # batch groups (2 batches each)
GB = BC // NG   # batches per group = 2
NSG = NT * GB   # 14 psum slots per group


def _bc_(v, pos, n):
    """Insert a zero-stride (broadcast) dim of length n at position pos."""
    ap = [list(x) for x in v.ap]
    ap.insert(pos, [0, n])
    return bass.AP(tensor=v.tensor, offset=v.offset, ap=ap)


def _ap(v, offset_delta, ap_list):
    return bass.AP(tensor=v.tensor, offset=v.offset + offset_delta, ap=ap_list)


def _body(nc, hist_in, cand_in, maskT_in, pos_in, pos_emb16, pos_emb32,
          posT16, w1t, b1_16, w2_16, ur_out, cand_out, tc, HP):
    import contextlib

    HW = 32 if HP <= 32 else 64          # psum h-padding per n-local
    NWIN = (8 * HW) // 128               # transpose windows per slot row
    PARS = 128 // HW                     # n-locals per window

    ctx = contextlib.ExitStack()
    with ctx:
        consts = ctx.enter_context(tc.tile_pool(name="consts", bufs=1))
        ps = ctx.enter_context(tc.tile_pool(name="ps", bufs=4, space="PSUM"))
        psmv = ctx.enter_context(tc.tile_pool(name="psmv", bufs=2, space="PSUM"))
        psat = ctx.enter_context(tc.tile_pool(name="psat", bufs=2, space="PSUM"))
        hidp = ctx.enter_context(tc.tile_pool(name="hid", bufs=4))
        exp_ = ctx.enter_context(tc.tile_pool(name="exp", bufs=2))

        # ---------------- constants ----------------
        ident16 = consts.tile([128, 128], BF)
        make_identity(nc, ident16)
        ident32 = consts.tile([128, 128], DT)
        make_identity(nc, ident32)

        # W1T[f, a] in 10 f-chunks of 100 (host provides W1 transposed, bf16)
        w1T = consts.tile([100, 10, A], BF)
        nc.sync.dma_start(out=w1T,
                          in_=w1t.ap().rearrange("(k p) a -> p k a", p=100))

        pos_emb_s = consts.tile([J, P], BF)
        nc.gpsimd.dma_start(out=pos_emb_s, in_=pos_emb16.ap())
        posT = consts.tile([P, J], BF)
        nc.gpsimd.dma_start(out=posT, in_=posT16.ap())
        maskT_s = consts.tile([HP, BC], DT)
        nc.gpsimd.dma_start(out=maskT_s, in_=maskT_in.ap())
        b1row = consts.tile([1, A], BF)
        nc.gpsimd.dma_start(out=b1row, in_=_ap(b1_16.ap(), 0, [[0, 1], [1, A]]))
        one11 = consts.tile([1, 1], BF)
        nc.vector.memset(one11, 1.0)

        # w2 chunks as columns + shifted zero-window tensors for slot matvec
        w2colA = consts.tile([A0, 1], BF)
        nc.gpsimd.dma_start(out=w2colA, in_=_ap(w2_16.ap(), 0, [[1, A0], [1, 1]]))
        w2colB = consts.tile([A1, 1], BF)
        nc.gpsimd.dma_start(out=w2colB, in_=_ap(w2_16.ap(), A0, [[1, A1], [1, 1]]))
        ZA = consts.tile([A0, 2 * NSG - 1], BF)
        ZB = consts.tile([A1, 2 * NSG - 1], BF)
        nc.vector.memset(ZA, 0.0)
        nc.vector.memset(ZB, 0.0)
        nc.vector.tensor_copy(out=ZA[:, NSG - 1:NSG], in_=w2colA)
        nc.vector.tensor_copy(out=ZB[:, NSG - 1:NSG], in_=w2colB)

        # E[j, a] = pos_emb @ Wh2.T  (Wh2 = W1[:, 900:1000] rows of w1t)
        E_s = consts.tile([J, A], BF)
        psE = ps.tile([J, A], DT, tag="ps")
        nc.tensor.matmul(psE, lhsT=posT[:, :], rhs=w1T[:, 9, :],
                         start=True, stop=True)
        nc.scalar.copy(out=E_s, in_=psE)

        # c0[a] = Wc2 @ pos_emb[1] + b1 as per-partition bias columns
        c0colA = consts.tile([A0, 1], DT)
        c0colB = consts.tile([A1, 1], DT)
        for c0c, asl in ((c0colA, slice(0, A0)), (c0colB, slice(A0, A))):
            psc = ps.tile([c0c.shape[0], 1], DT, tag="ps")
            nc.tensor.matmul(psc, lhsT=w1T[:, 4, asl], rhs=posT[:, 1:2],
                             start=True, stop=False)
            nc.tensor.matmul(psc, lhsT=b1row[:, asl], rhs=one11[:, :],
                             start=False, stop=True)
            nc.scalar.copy(out=c0c, in_=psc)

        # one-hot of positions, transposed: onehot[j, b*HP+h] = (pos[b,h]==j)
        pos52 = consts.tile([J, BC * HP], I32)
        nc.gpsimd.dma_start(out=pos52, in_=_bc_(pos_in.ap(), 0, J))
        iot = consts.tile([J, BC * HP], I32)
        nc.gpsimd.iota(iot, pattern=[[0, BC * HP]], base=0, channel_multiplier=1)
        onehot = consts.tile([J, BC * HP], BF)
        nc.vector.tensor_tensor(out=onehot, in0=iot, in1=pos52, op=ALU.is_equal)

        # ---------------- full-input loads --------------------------------
        cand_all = consts.tile([100, 4, D], DT)   # [2x50 rows, batch-pair, feat]
        hist_all = consts.tile([2 * HP, 4, D], DT)
        for hf in range(2):
            src_c = _ap(cand_in.ap(), hf * N * D,
                        [[D, 50], [2 * N * D, 4], [1, D]])
            src_h = _ap(hist_in.ap(), hf * HP * D,
                        [[D, HP], [2 * HP * D, 4], [1, D]])
            nc.sync.dma_start(out=cand_all[hf * 50:(hf + 1) * 50, :, :],
                              in_=src_c)
            nc.sync.dma_start(out=hist_all[hf * HP:(hf + 1) * HP, :, :],
                              in_=src_h)

        # candidate passthrough: two strided DMAs + pos_emb[1] broadcast
        for hf in range(2):
            dst = _ap(cand_out.ap(), hf * N * F,
                      [[F, 50], [2 * N * F, 4], [1, D]])
            nc.sync.dma_start(out=dst, in_=cand_all[hf * 50:(hf + 1) * 50, :, :])
        nc.gpsimd.dma_start(
            out=cand_out.ap()[:, :, D:F],
            in_=_bc_(_bc_(pos_emb32.ap()[1:2, :], 0, N), 0, BC))

        # hist rows re-staged [h, b, f] for the attention V matrix
        staging = consts.tile([HP, BC, D], DT)
        src_hf = _ap(hist_in.ap(), 0, [[D, HP], [HP * D, BC], [1, D]])
        nc.sync.dma_start(out=staging, in_=src_hf)

        # ---------------- persistent SBUF tensors -------------------------
        candT = consts.tile([100, 4, BC * N], BF)  # [feat-chunk, k, (b,n)]
        histT = consts.tile([100, 4, BC * HP], BF)
        hcT2A = consts.tile([A0, BC * N, 2], BF)
        hcT2B = consts.tile([A1, BC * N, 2], BF)
        neghhA = consts.tile([A0, BC * HP], BF)
        neghhB = consts.tile([A1, BC * HP], BF)
        mw = consts.tile([HP, BC], DT)
        histf16 = consts.tile([HP, BC, F + 1], BF)
        alphaT = consts.tile([HP, BC, 64], BF)
        urs = consts.tile([N, BC, F], DT)
        rs_all = consts.tile([N, BC], DT)

        # ---------------- transposes (all groups) + wide GEMMs ------------
        for g in range(4):
            ptc = ps.tile([100, 4, 100], DT, tag="ps")
            pth = ps.tile([100, 4, 2 * HP], DT, tag="ps")
            for k in range(4):
                nc.tensor.transpose(
                    ptc[:, k, :], cand_all[:, g, k * 100:(k + 1) * 100],
                    ident32[:100, :100])
                nc.tensor.transpose(
                    pth[:, k, :], hist_all[:, g, k * 100:(k + 1) * 100],
                    ident32[:2 * HP, :2 * HP])
            nc.scalar.copy(out=candT[:, :, g * 100:(g + 1) * 100], in_=ptc)
            nc.scalar.copy(out=histT[:, :, g * 2 * HP:(g + 1) * 2 * HP],
                           in_=pth)

        for asl, hcT2, neghh, c0c in (
                (slice(0, A0), hcT2A, neghhA, c0colA),
                (slice(A0, A), hcT2B, neghhB, c0colB)):
            m = asl.stop - asl.start
            pg = ps.tile([m, BC * N], DT, tag="ps")
            for k in range(4):
                nc.tensor.matmul(pg, lhsT=w1T[:, k, asl], rhs=candT[:, k, :],
                                 start=(k == 0), stop=(k == 3))
            nc.scalar.activation(out=hcT2, in_=_bc_(pg[:, :], 2, 2),
                                 func=AF.Identity, bias=c0c, scale=1.0)
            ph = ps.tile([m, BC * HP], DT, tag="ps")
            for k in range(4):
                nc.tensor.matmul(ph, lhsT=w1T[:, 5 + k, asl],
                                 rhs=histT[:, k, :],
                                 start=(k == 0), stop=False)
            nc.tensor.matmul(ph, lhsT=E_s[:, asl], rhs=onehot[:, :],
                             start=False, stop=True)
            nc.scalar.activation(out=neghh, in_=ph, func=AF.Copy, scale=-1.0)

        # ---------------- per-group prep: shT + mw + histf ----------------
        def head(g):
            psh = ps.tile([HP, GB], DT, tag="ps")
            for bl in range(GB):
                b = g * GB + bl
                hsl = slice(b * HP, (b + 1) * HP)
                nc.tensor.matmul(psh[:, bl:bl + 1], lhsT=neghhA[:, hsl],
                                 rhs=w2colA, start=True, stop=False)
                nc.tensor.matmul(psh[:, bl:bl + 1], lhsT=neghhB[:, hsl],
                                 rhs=w2colB, start=False, stop=True)
            shE = exp_.tile([HP, GB], DT, tag="she")
            nc.scalar.activation(out=shE, in_=psh, func=AF.Exp, scale=-1.0)
            bsl = slice(g * GB, (g + 1) * GB)
            nc.vector.tensor_tensor(out=mw[:, bsl], in0=shE,
                                    in1=maskT_s[:, bsl], op=ALU.mult)

            # histf (V matrix) with m-scale + m column (ssum source);
            # pos-emb part straight from psum.
            for bl in range(GB):
                b = g * GB + bl
                nc.scalar.activation(out=histf16[:, b, 0:D],
                                     in_=staging[:, b, :],
                                     func=AF.Copy, scale=mw[:, b:b + 1])
                ppg = ps.tile([HP, P], DT, tag="ps")
                nc.tensor.matmul(ppg, lhsT=onehot[:, b * HP:(b + 1) * HP],
                                 rhs=pos_emb_s[:, :], start=True, stop=True)
                nc.scalar.activation(out=histf16[:, b, D:F], in_=ppg,
                                     func=AF.Copy, scale=mw[:, b:b + 1])
                nc.scalar.copy(out=histf16[:, b, F:F + 1], in_=mw[:, b:b + 1])

        # ---------------- per-group matvec (max pass + slot matmuls) ------
        def matvec(g):
            pmv = psmv.tile([NSG, 8, HW], DT, tag="pmv")
            first = True
            for bl in range(GB):
                b = g * GB + bl
                nsl = slice(b * N, (b + 1) * N)
                hids = []
                for hcT2, neghh, aw in ((hcT2A, neghhA, A0),
                                        (hcT2B, neghhB, A1)):
                    hid = hidp.tile([aw, N * HP], BF, tag=f"hid{aw}")
                    v = hcT2[:, nsl, :]
                    hcb = _bc_(v, 2, HP // 2)           # [aw, 50, HP/2, 2]
                    w = neghh[:, b * HP:(b + 1) * HP]
                    hhb = _ap(w, 0, [list(w.ap[0]), [0, N], [2, HP // 2],
                                     [1, 2]])
                    hidv = _ap(hid, 0, [list(hid.ap[0]), [HP, N],
                                        [2, HP // 2], [1, 2]])
                    nc.vector.tensor_tensor(out=hidv, in0=hcb, in1=hhb,
                                            op=ALU.max)
                    hids.append(hid)
                for t in range(NT):
                    s = t * GB + bl
                    nn = 8 if t < 6 else 2   # n-cols in this tile
                    rsl = slice(t * 8 * HP, (t * 8 + nn) * HP)
                    dst = _ap(pmv, 0, [list(pmv.ap[0]), [HW, nn], [1, HP]])
                    for Z, hv in ((ZA, hids[0]), (ZB, hids[1])):
                        last = (bl == GB - 1 and t == NT - 1 and Z is ZB)
                        nc.tensor.matmul(
                            dst, lhsT=Z[:, NSG - 1 - s:2 * NSG - 1 - s],
                            rhs=hv[:, rsl], start=first, stop=last)
                        first = False
            return pmv

        # ---------------- per-group tail: exp -> alpha -> attention -------
        def tail(g, pmv):
            ex = exp_.tile([NSG, 8, HW], BF, tag="ex")
            nc.scalar.activation(out=ex[:, :, 0:HP], in_=pmv[:, :, 0:HP],
                                 func=AF.Exp)
            ptr = ps.tile([128, NWIN, NSG], BF, tag="ps")
            for w in range(NWIN):
                nc.tensor.transpose(
                    ptr[:, w, :],
                    _ap(ex, w * 128, [list(ex.ap[0]), [1, 128]]),
                    ident16[:NSG, :NSG])
            exT = exp_.tile([128, NWIN, NSG], BF, tag="exT")
            nc.scalar.copy(out=exT, in_=ptr)
            # alpha assembly on Pool: alphaT[h, b, n], n = 8t + PARS*w + par
            # (one copy per par covers all windows w)
            for par in range(PARS):
                src = exT[par * HW:par * HW + HP, :, :]
                nc.gpsimd.tensor_copy(
                    out=_ap(alphaT, g * GB * 64 + par,
                            [list(alphaT.ap[0]), [PARS, NWIN], [8, NT],
                             [64, GB]]),
                    in_=_ap(src, 0, [list(src.ap[0]), [NSG, NWIN], [GB, NT],
                                     [1, GB]]))
            # attention: ur = alpha @ histf / ssum
            for bl in range(GB):
                b = g * GB + bl
                pur = psat.tile([N, F + 1], DT, tag="pur")
                nc.tensor.matmul(pur, lhsT=alphaT[:, b, 0:N],
                                 rhs=histf16[:, b, :], start=True, stop=True)
                nc.vector.reciprocal(rs_all[:, b:b + 1], pur[:, F:F + 1])
                nc.scalar.activation(out=urs[:, b, :], in_=pur[:, 0:F],
                                     func=AF.Copy, scale=rs_all[:, b:b + 1])
            dst_ur = _ap(ur_out.ap(), g * GB * N * F,
                         [[F, N], [N * F, GB], [1, F]])
            nc.sync.dma_start(out=dst_ur, in_=urs[:, g * GB:(g + 1) * GB, :])

        # ---------------- software pipeline ------------------------------
        head(0)
        head(1)
        pmv0 = matvec(0)
        head(2)
        pmv1 = matvec(1)
        tail(0, pmv0)
        head(3)
        pmv2 = matvec(2)
        tail(1, pmv1)
        pmv3 = matvec(3)
        tail(2, pmv2)
        tail(3, pmv3)


def build(debug=False, reps=1, loop=1, HP=H):
    nc = bacc.Bacc("TRN2", target_bir_lowering=False, debug=debug)
    hist_in = nc.dram_tensor("hist_in", [BC, HP, D], DT, kind="ExternalInput")
    cand_in = nc.dram_tensor("cand_in", [BC, N, D], DT, kind="ExternalInput")
    maskT_in = nc.dram_tensor("maskT_in", [HP, BC], DT, kind="ExternalInput")
    pos_in = nc.dram_tensor("pos_in", [BC, HP], I32, kind="ExternalInput")
    pos_emb16 = nc.dram_tensor("pos_emb16", [J, P], BF, kind="ExternalInput")
    pos_emb32 = nc.dram_tensor("pos_emb32", [J, P], DT, kind="ExternalInput")
    posT16 = nc.dram_tensor("posT16", [P, J], BF, kind="ExternalInput")
    w1t = nc.dram_tensor("w1t", [2 * F, A], BF, kind="ExternalInput")
    b1_16 = nc.dram_tensor("b1_16", [A], BF, kind="ExternalInput")
    w2_16 = nc.dram_tensor("w2_16", [A], BF, kind="ExternalInput")
    ur_out = nc.dram_tensor("ur_out", [BC, N, F], DT, kind="ExternalOutput")
    cand_out = nc.dram_tensor("cand_out", [BC, N, F], DT, kind="ExternalOutput")

    with tile.TileContext(nc) as tc:
        if loop > 1:
            with tc.For_i(0, loop):
                _body(nc, hist_in, cand_in, maskT_in, pos_in, pos_emb16,
                      pos_emb32, posT16, w1t, b1_16, w2_16, ur_out,
                      cand_out, tc, HP)
        else:
            for _ in range(reps):
                _body(nc, hist_in, cand_in, maskT_in, pos_in, pos_emb16,
                      pos_emb32, posT16, w1t, b1_16, w2_16, ur_out,
                      cand_out, tc, HP)
    nc.compile()
    return nc


_NCS = {}


def _get_nc(HP):
    if HP not in _NCS:
        _NCS[HP] = build(debug=False, HP=HP)
    return _NCS[HP]


def _bf(x):
    return np.ascontiguousarray(np.asarray(x, np.float32).astype(
        ml_dtypes.bfloat16))


def choose_hp(user_history_mask):
    k = int(np.asarray(user_history_mask).astype(bool).sum(axis=1).max())
    hp = max(2, k + (k % 2))
    return min(hp, H)


def compact(history_repr, user_history_mask, user_history_position, HP):
    """Gather kept (mask=1) history rows, pad to HP; pad rows get mask 0."""
    hist = np.asarray(history_repr, np.float32)
    mask = np.asarray(user_history_mask).astype(bool)
    pos = np.asarray(user_history_position).astype(np.int32)
    Bn = hist.shape[0]
    hist_g = np.zeros((Bn, HP, hist.shape[2]), np.float32)
    pos_g = np.zeros((Bn, HP), np.int32)
    mask_g = np.zeros((Bn, HP), np.float32)
    for b in range(Bn):
        idx = np.flatnonzero(mask[b])[:HP]
        k = len(idx)
        hist_g[b, :k] = hist[b, idx]
        pos_g[b, :k] = pos[b, idx]
        mask_g[b, :k] = 1.0
    return hist_g, pos_g, mask_g


def make_in_maps(history_repr, candidate_repr, user_history_mask,
                 user_history_position, pos_emb, W1, b1, w2, HP=None):
    if HP is None:
        HP = choose_hp(user_history_mask)
    hist_g, pos_g, mask_g = compact(history_repr, user_history_mask,
                                    user_history_position, HP)
    cand = np.ascontiguousarray(np.asarray(candidate_repr, np.float32))
    pe32 = np.ascontiguousarray(np.asarray(pos_emb, np.float32))
    pe16 = _bf(pe32)
    peT16 = _bf(pe32.T)
    w1t16 = _bf(np.asarray(W1, np.float32).T)
    b1_16 = _bf(b1)
    w2_16 = _bf(w2)
    in_maps = []
    for c in range(NCORES):
        sl = slice(c * BC, (c + 1) * BC)
        in_maps.append({
            "hist_in": np.ascontiguousarray(hist_g[sl]),
            "cand_in": cand[sl],
            "maskT_in": np.ascontiguousarray(mask_g[sl].T),
            "pos_in": np.ascontiguousarray(pos_g[sl]),
            "pos_emb16": pe16, "pos_emb32": pe32, "posT16": peT16,
            "w1t": w1t16, "b1_16": b1_16, "w2_16": w2_16,
        })
    return in_maps


def kernel(history_repr, candidate_repr, user_history_mask,
           user_history_position, pos_emb, W1, b1, w2, b2=None, **_ignored):
    # b2 shifts every logit equally -> cancels in softmax; unused.
    HP = choose_hp(user_history_mask)
    nc = _get_nc(HP)
    in_maps = make_in_maps(history_repr, candidate_repr, user_history_mask,
                           user_history_position, pos_emb, W1, b1, w2, HP=HP)
    res = bass_utils.run_bass_kernel_spmd(nc, in_maps, list(range(NCORES)))
    ur = np.concatenate([res.results[c]["ur_out"] for c in range(NCORES)], 0)
    cand = np.concatenate([res.results[c]["cand_out"] for c in range(NCORES)], 0)
    return ur, cand


# revision 17
# speedup vs baseline: 1.2449x; 1.0286x over previous
"""Trainium2 Bass kernel for the news-attention module (v5).

Math (exact, unchanged):
  relu(hc + hh) = max(hc, -hh) + hh
  logits a[n,h] = w2 . max(hc[n], -hh[h]) + sh[h],  sh = w2 . hh
  softmax(a + maskbias) = exp(w2.max-part) * m[h] / sum,
      m[h] = mask[h] * exp(sh[h])
  ur[n] = (1/ssum) sum_h exnorm[n,h] * (m[h] * histf[h])

v5 = v4 pipeline + masked-history compaction: rows with mask=0
contribute exactly 0 (m[h]=0), so the host gathers only kept rows and
pads each batch to HP (= even ceil of the max kept count, <= 50); pad
rows get mask 0 and behave identically to masked rows. The entire
history pipeline (hh GEMM, max pass, matvec, attention K) runs at HP
instead of 50. The module is compiled per HP on first use.

Sharding: data-parallel over batch, 8 batches per core on 8 cores.
"""

import sys

for _p in ("/opt/trn_rl_repo",):
    if _p not in sys.path:
        sys.path.insert(0, _p)

import numpy as np
import ml_dtypes

import concourse.bass as bass
import concourse.bacc as bacc
import concourse.tile as tile
from concourse import mybir
from concourse import bass_utils
from concourse.masks import make_identity

DT = mybir.dt.float32
BF = mybir.dt.bfloat16
I32 = mybir.dt.int32
AF = mybir.ActivationFunctionType
ALU = mybir.AluOpType

NCORES = 8
B = 64
BC = B // NCORES  # 8 batches per core
H = 50
N = 50
D = 400
P = 100
A = 200
F = D + P       # 500
J = 52
A0 = 128        # a-chunk split: [0:128), [128:200)
A1 = A - A0     # 72
NT = 7          # matvec n-tiles per batch (8 n-locals each)
NG = 4          # batch groups (2 batches each)
GB = BC // NG   # batches per group = 2
NSG = NT * GB   # 14 psum slots per group


def _bc_(v, pos, n):
    """Insert a zero-stride (broadcast) dim of length n at position pos."""
    ap = [list(x) for x in v.ap]
    ap.insert(pos, [0, n])
    return bass.AP(tensor=v.tensor, offset=v.offset, ap=ap)


def _ap(v, offset_delta, ap_list):
    return bass.AP(tensor=v.tensor, offset=v.offset + offset_delta, ap=ap_list)


def _body(nc, hist_in, cand_in, maskT_in, pos_in, pos_emb16, pos_emb32,
          posT16, w1t, b1_16, w2_16, ur_out, cand_out, tc, HP):
    import contextlib

    HW = 32 if HP <= 32 else 64          # psum h-padding per n-local
    NWIN = (8 * HW) // 128               # transpose windows per slot row
    PARS = 128 // HW                     # n-locals per window

    ctx = contextlib.ExitStack()
    with ctx:
        consts = ctx.enter_context(tc.tile_pool(name="consts", bufs=1))
        ps = ctx.enter_context(tc.tile_pool(name="ps", bufs=4, space="PSUM"))
        psmv = ctx.enter_context(tc.tile_pool(name="psmv", bufs=2, space="PSUM"))
        psat = ctx.enter_context(tc.tile_pool(name="psat", bufs=2, space="PSUM"))
        hidp = ctx.enter_context(tc.tile_pool(name="hid", bufs=4))
        exp_ = ctx.enter_context(tc.tile_pool(name="exp", bufs=2))

        # ---------------- constants ----------------
        ident16 = consts.tile([128, 128], BF)
        make_identity(nc, ident16)
        ident32 = consts.tile([128, 128], DT)
        make_identity(nc, ident32)

        # W1T[f, a] in 10 f-chunks of 100 (host provides W1 transposed, bf16)
        w1T = consts.tile([100, 10, A], BF)
        nc.sync.dma_start(out=w1T,
                          in_=w1t.ap().rearrange("(k p) a -> p k a", p=100))

        pos_emb_s = consts.tile([J, P], BF)
        nc.gpsimd.dma_start(out=pos_emb_s, in_=pos_emb16.ap())
        posT = consts.tile([P, J], BF)
        nc.gpsimd.dma_start(out=posT, in_=posT16.ap())
        maskT_s = consts.tile([HP, BC], DT)
        nc.gpsimd.dma_start(out=maskT_s, in_=maskT_in.ap())
        b1row = consts.tile([1, A], BF)
        nc.gpsimd.dma_start(out=b1row, in_=_ap(b1_16.ap(), 0, [[0, 1], [1, A]]))
        one11 = consts.tile([1, 1], BF)
        nc.vector.memset(one11, 1.0)

        # w2 chunks as columns + shifted zero-window tensors for slot matvec
        w2colA = consts.tile([A0, 1], BF)
        nc.gpsimd.dma_start(out=w2colA, in_=_ap(w2_16.ap(), 0, [[1, A0], [1, 1]]))
        w2colB = consts.tile([A1, 1], BF)
        nc.gpsimd.dma_start(out=w2colB, in_=_ap(w2_16.ap(), A0, [[1, A1], [1, 1]]))
        ZA = consts.tile([A0, 2 * NSG - 1], BF)
        ZB = consts.tile([A1, 2 * NSG - 1], BF)
        nc.vector.memset(ZA, 0.0)
        nc.vector.memset(ZB, 0.0)
        nc.vector.tensor_copy(out=ZA[:, NSG - 1:NSG], in_=w2colA)
        nc.vector.tensor_copy(out=ZB[:, NSG - 1:NSG], in_=w2colB)

        # E[j, a] = pos_emb @ Wh2.T  (Wh2 = W1[:, 900:1000] rows of w1t)
        E_s = consts.tile([J, A], BF)
        psE = ps.tile([J, A], DT, tag="ps")
        nc.tensor.matmul(psE, lhsT=posT[:, :], rhs=w1T[:, 9, :],
                         start=True, stop=True)
        nc.scalar.copy(out=E_s, in_=psE)

        # c0[a] = Wc2 @ pos_emb[1] + b1 as per-partition bias columns
        c0colA = consts.tile([A0, 1], DT)
        c0colB = consts.tile([A1, 1], DT)
        for c0c, asl in ((c0colA, slice(0, A0)), (c0colB, slice(A0, A))):
            psc = ps.tile([c0c.shape[0], 1], DT, tag="ps")
            nc.tensor.matmul(psc, lhsT=w1T[:, 4, asl], rhs=posT[:, 1:2],
                             start=True, stop=False)
            nc.tensor.matmul(psc, lhsT=b1row[:, asl], rhs=one11[:, :],
                             start=False, stop=True)
            nc.scalar.copy(out=c0c, in_=psc)

        # one-hot of positions, transposed: onehot[j, b*HP+h] = (pos[b,h]==j)
        pos52 = consts.tile([J, BC * HP], I32)
        nc.gpsimd.dma_start(out=pos52, in_=_bc_(pos_in.ap(), 0, J))
        iot = consts.tile([J, BC * HP], I32)
        nc.gpsimd.iota(iot, pattern=[[0, BC * HP]], base=0, channel_multiplier=1)
        onehot = consts.tile([J, BC * HP], BF)
        nc.vector.tensor_tensor(out=onehot, in0=iot, in1=pos52, op=ALU.is_equal)

        # ---------------- full-input loads --------------------------------
        cand_all = consts.tile([100, 4, D], DT)   # [2x50 rows, batch-pair, feat]
        hist_all = consts.tile([2 * HP, 4, D], DT)
        for hf in range(2):
            src_c = _ap(cand_in.ap(), hf * N * D,
                        [[D, 50], [2 * N * D, 4], [1, D]])
            src_h = _ap(hist_in.ap(), hf * HP * D,
                        [[D, HP], [2 * HP * D, 4], [1, D]])
            nc.sync.dma_start(out=cand_all[hf * 50:(hf + 1) * 50, :, :],
                              in_=src_c)
            nc.sync.dma_start(out=hist_all[hf * HP:(hf + 1) * HP, :, :],
                              in_=src_h)

        # candidate passthrough: two strided DMAs + pos_emb[1] broadcast
        for hf in range(2):
            for p in range(4):
                dst = _ap(cand_out.ap(), hf * N * F + p * 2 * N * F,
                          [[F, 50], [1, D]])
                nc.sync.dma_start(
                    out=dst, in_=cand_all[hf * 50:(hf + 1) * 50, p, :])
        nc.gpsimd.dma_start(
            out=cand_out.ap()[:, :, D:F],
            in_=_bc_(_bc_(pos_emb32.ap()[1:2, :], 0, N), 0, BC))

        # hist rows re-staged [h, b, f] for the attention V matrix
        staging = consts.tile([HP, BC, D], DT)
        src_hf = _ap(hist_in.ap(), 0, [[D, HP], [HP * D, BC], [1, D]])
        nc.sync.dma_start(out=staging, in_=src_hf)

        # ---------------- persistent SBUF tensors -------------------------
        candT = consts.tile([100, 4, BC * N], BF)  # [feat-chunk, k, (b,n)]
        histT = consts.tile([100, 4, BC * HP], BF)
        hcT2A = consts.tile([A0, BC * N, 2], BF)
        hcT2B = consts.tile([A1, BC * N, 2], BF)
        neghhA = consts.tile([A0, BC * HP], BF)
        neghhB = consts.tile([A1, BC * HP], BF)
        mw = consts.tile([HP, BC], DT)
        histf16 = consts.tile([HP, BC, F + 1], BF)
        alphaT = consts.tile([HP, BC, 64], BF)
        urs = consts.tile([N, BC, F], DT)
        rs_all = consts.tile([N, BC], DT)

        # ---------------- transposes (all groups) + wide GEMMs ------------
        for g in range(4):
            ptc = ps.tile([100, 4, 100], DT, tag="ps")
            pth = ps.tile([100, 4, 2 * HP], DT, tag="ps")
            for k in range(4):
                nc.tensor.transpose(
                    ptc[:, k, :], cand_all[:, g, k * 100:(k + 1) * 100],
                    ident32[:100, :100])
                nc.tensor.transpose(
                    pth[:, k, :], hist_all[:, g, k * 100:(k + 1) * 100],
                    ident32[:2 * HP, :2 * HP])
            nc.scalar.copy(out=candT[:, :, g * 100:(g + 1) * 100], in_=ptc)
            nc.scalar.copy(out=histT[:, :, g * 2 * HP:(g + 1) * 2 * HP],
                           in_=pth)

        for asl, hcT2, neghh, c0c in (
                (slice(0, A0), hcT2A, neghhA, c0colA),
                (slice(A0, A), hcT2B, neghhB, c0colB)):
            m = asl.stop - asl.start
            pg = ps.tile([m, BC * N], DT, tag="ps")
            for k in range(4):
                nc.tensor.matmul(pg, lhsT=w1T[:, k, asl], rhs=candT[:, k, :],
                                 start=(k == 0), stop=(k == 3))
            nc.scalar.activation(out=hcT2, in_=_bc_(pg[:, :], 2, 2),
                                 func=AF.Identity, bias=c0c, scale=1.0)
            ph = ps.tile([m, BC * HP], DT, tag="ps")
            for k in range(4):
                nc.tensor.matmul(ph, lhsT=w1T[:, 5 + k, asl],
                                 rhs=histT[:, k, :],
                                 start=(k == 0), stop=False)
            nc.tensor.matmul(ph, lhsT=E_s[:, asl], rhs=onehot[:, :],
                             start=False, stop=True)
            nc.scalar.activation(out=neghh, in_=ph, func=AF.Copy, scale=-1.0)

        # ---------------- per-group prep: shT + mw + histf ----------------
        def head(g):
            psh = ps.tile([HP, GB], DT, tag="ps")
            for bl in range(GB):
                b = g * GB + bl
                hsl = slice(b * HP, (b + 1) * HP)
                nc.tensor.matmul(psh[:, bl:bl + 1], lhsT=neghhA[:, hsl],
                                 rhs=w2colA, start=True, stop=False)
                nc.tensor.matmul(psh[:, bl:bl + 1], lhsT=neghhB[:, hsl],
                                 rhs=w2colB, start=False, stop=True)
            shE = exp_.tile([HP, GB], DT, tag="she")
            nc.scalar.activation(out=shE, in_=psh, func=AF.Exp, scale=-1.0)
            bsl = slice(g * GB, (g + 1) * GB)
            nc.vector.tensor_tensor(out=mw[:, bsl], in0=shE,
                                    in1=maskT_s[:, bsl], op=ALU.mult)

            # histf (V matrix) with m-scale + m column (ssum source);
            # pos-emb part straight from psum.
            for bl in range(GB):
                b = g * GB + bl
                nc.scalar.activation(out=histf16[:, b, 0:D],
                                     in_=staging[:, b, :],
                                     func=AF.Copy, scale=mw[:, b:b + 1])
                ppg = ps.tile([HP, P], DT, tag="ps")
                nc.tensor.matmul(ppg, lhsT=onehot[:, b * HP:(b + 1) * HP],
                                 rhs=pos_emb_s[:, :], start=True, stop=True)
                nc.scalar.activation(out=histf16[:, b, D:F], in_=ppg,
                                     func=AF.Copy, scale=mw[:, b:b + 1])
                nc.scalar.copy(out=histf16[:, b, F:F + 1], in_=mw[:, b:b + 1])

        # ---------------- per-group matvec (max pass + slot matmuls) ------
        def matvec(g):
            pmv = psmv.tile([NSG, 8, HW], DT, tag="pmv")
            first = True
            for bl in range(GB):
                b = g * GB + bl
                nsl = slice(b * N, (b + 1) * N)
                hids = []
                for hcT2, neghh, aw in ((hcT2A, neghhA, A0),
                                        (hcT2B, neghhB, A1)):
                    hid = hidp.tile([aw, N * HP], BF, tag=f"hid{aw}")
                    v = hcT2[:, nsl, :]
                    hcb = _bc_(v, 2, HP // 2)           # [aw, 50, HP/2, 2]
                    w = neghh[:, b * HP:(b + 1) * HP]
                    hhb = _ap(w, 0, [list(w.ap[0]), [0, N], [2, HP // 2],
                                     [1, 2]])
                    hidv = _ap(hid, 0, [list(hid.ap[0]), [HP, N],
                                        [2, HP // 2], [1, 2]])
                    nc.vector.tensor_tensor(out=hidv, in0=hcb, in1=hhb,
                                            op=ALU.max)
                    hids.append(hid)
                for t in range(NT):
                    s = t * GB + bl
                    nn = 8 if t < 6 else 2   # n-cols in this tile
                    rsl = slice(t * 8 * HP, (t * 8 + nn) * HP)
                    dst = _ap(pmv, 0, [list(pmv.ap[0]), [HW, nn], [1, HP]])
                    for Z, hv in ((ZA, hids[0]), (ZB, hids[1])):
                        last = (bl == GB - 1 and t == NT - 1 and Z is ZB)
                        nc.tensor.matmul(
                            dst, lhsT=Z[:, NSG - 1 - s:2 * NSG - 1 - s],
                            rhs=hv[:, rsl], start=first, stop=last)
                        first = False
            return pmv

        # ---------------- per-group tail: exp -> alpha -> attention -------
        def tail(g, pmv):
            ex = exp_.tile([NSG, 8, HW], BF, tag="ex")
            nc.scalar.activation(out=ex[:, :, 0:HP], in_=pmv[:, :, 0:HP],
                                 func=AF.Exp)
            ptr = ps.tile([128, NWIN, NSG], BF, tag="ps")
            for w in range(NWIN):
                nc.tensor.transpose(
                    ptr[:, w, :],
                    _ap(ex, w * 128, [list(ex.ap[0]), [1, 128]]),
                    ident16[:NSG, :NSG])
            exT = exp_.tile([128, NWIN, NSG], BF, tag="exT")
            nc.scalar.copy(out=exT, in_=ptr)
            # alpha assembly on Pool: alphaT[h, b, n], n = 8t + PARS*w + par
            # (one copy per par covers all windows w)
            for par in range(PARS):
                src = exT[par * HW:par * HW + HP, :, :]
                nc.gpsimd.tensor_copy(
                    out=_ap(alphaT, g * GB * 64 + par,
                            [list(alphaT.ap[0]), [PARS, NWIN], [8, NT],
                             [64, GB]]),
                    in_=_ap(src, 0, [list(src.ap[0]), [NSG, NWIN], [GB, NT],
                                     [1, GB]]))
            # attention: ur = alpha @ histf / ssum
            for bl in range(GB):
                b = g * GB + bl
                pur = psat.tile([N, F + 1], DT, tag="pur")
                nc.tensor.matmul(pur, lhsT=alphaT[:, b, 0:N],
                                 rhs=histf16[:, b, :], start=True, stop=True)
                nc.vector.reciprocal(rs_all[:, b:b + 1], pur[:, F:F + 1])
                nc.scalar.activation(out=urs[:, b, :], in_=pur[:, 0:F],
                                     func=AF.Copy, scale=rs_all[:, b:b + 1])
            for bl in range(GB):
                b = g * GB + bl
                dst_ur = _ap(ur_out.ap(), b * N * F, [[F, N], [1, F]])
                nc.sync.dma_start(out=dst_ur, in_=urs[:, b, :])

        # ---------------- software pipeline ------------------------------
        head(0)
        head(1)
        pmv0 = matvec(0)
        head(2)
        pmv1 = matvec(1)
        tail(0, pmv0)
        head(3)
        pmv2 = matvec(2)
        tail(1, pmv1)
        pmv3 = matvec(3)
        tail(2, pmv2)
        tail(3, pmv3)


def build(debug=False, reps=1, loop=1, HP=H):
    nc = bacc.Bacc("TRN2", target_bir_lowering=False, debug=debug)
    hist_in = nc.dram_tensor("hist_in", [BC, HP, D], DT, kind="ExternalInput")
    cand_in = nc.dram_tensor("cand_in", [BC, N, D], DT, kind="ExternalInput")
    maskT_in = nc.dram_tensor("maskT_in", [HP, BC], DT, kind="ExternalInput")
    pos_in = nc.dram_tensor("pos_in", [BC, HP], I32, kind="ExternalInput")
    pos_emb16 = nc.dram_tensor("pos_emb16", [J, P], BF, kind="ExternalInput")
    pos_emb32 = nc.dram_tensor("pos_emb32", [J, P], DT, kind="ExternalInput")
    posT16 = nc.dram_tensor("posT16", [P, J], BF, kind="ExternalInput")
    w1t = nc.dram_tensor("w1t", [2 * F, A], BF, kind="ExternalInput")
    b1_16 = nc.dram_tensor("b1_16", [A], BF, kind="ExternalInput")
    w2_16 = nc.dram_tensor("w2_16", [A], BF, kind="ExternalInput")
    ur_out = nc.dram_tensor("ur_out", [BC, N, F], DT, kind="ExternalOutput")
    cand_out = nc.dram_tensor("cand_out", [BC, N, F], DT, kind="ExternalOutput")

    with tile.TileContext(nc) as tc:
        if loop > 1:
            with tc.For_i(0, loop):
                _body(nc, hist_in, cand_in, maskT_in, pos_in, pos_emb16,
                      pos_emb32, posT16, w1t, b1_16, w2_16, ur_out,
                      cand_out, tc, HP)
        else:
            for _ in range(reps):
                _body(nc, hist_in, cand_in, maskT_in, pos_in, pos_emb16,
                      pos_emb32, posT16, w1t, b1_16, w2_16, ur_out,
                      cand_out, tc, HP)
    nc.compile()
    return nc


_NCS = {}


def _get_nc(HP):
    if HP not in _NCS:
        _NCS[HP] = build(debug=False, HP=HP)
    return _NCS[HP]


def _bf(x):
    return np.ascontiguousarray(np.asarray(x, np.float32).astype(
        ml_dtypes.bfloat16))


def choose_hp(user_history_mask):
    k = int(np.asarray(user_history_mask).astype(bool).sum(axis=1).max())
    hp = max(2, k + (k % 2))
    return min(hp, H)


def compact(history_repr, user_history_mask, user_history_position, HP):
    """Gather kept (mask=1) history rows, pad to HP; pad rows get mask 0."""
    hist = np.asarray(history_repr, np.float32)
    mask = np.asarray(user_history_mask).astype(bool)
    pos = np.asarray(user_history_position).astype(np.int32)
    Bn = hist.shape[0]
    hist_g = np.zeros((Bn, HP, hist.shape[2]), np.float32)
    pos_g = np.zeros((Bn, HP), np.int32)
    mask_g = np.zeros((Bn, HP), np.float32)
    for b in range(Bn):
        idx = np.flatnonzero(mask[b])[:HP]
        k = len(idx)
        hist_g[b, :k] = hist[b, idx]
        pos_g[b, :k] = pos[b, idx]
        mask_g[b, :k] = 1.0
    return hist_g, pos_g, mask_g


def make_in_maps(history_repr, candidate_repr, user_history_mask,
                 user_history_position, pos_emb, W1, b1, w2, HP=None):
    if HP is None:
        HP = choose_hp(user_history_mask)
    hist_g, pos_g, mask_g = compact(history_repr, user_history_mask,
                                    user_history_position, HP)
    cand = np.ascontiguousarray(np.asarray(candidate_repr, np.float32))
    pe32 = np.ascontiguousarray(np.asarray(pos_emb, np.float32))
    pe16 = _bf(pe32)
    peT16 = _bf(pe32.T)
    w1t16 = _bf(np.asarray(W1, np.float32).T)
    b1_16 = _bf(b1)
    w2_16 = _bf(w2)
    in_maps = []
    for c in range(NCORES):
        sl = slice(c * BC, (c + 1) * BC)
        in_maps.append({
            "hist_in": np.ascontiguousarray(hist_g[sl]),
            "cand_in": cand[sl],
            "maskT_in": np.ascontiguousarray(mask_g[sl].T),
            "pos_in": np.ascontiguousarray(pos_g[sl]),
            "pos_emb16": pe16, "pos_emb32": pe32, "posT16": peT16,
            "w1t": w1t16, "b1_16": b1_16, "w2_16": w2_16,
        })
    return in_maps


def kernel(history_repr, candidate_repr, user_history_mask,
           user_history_position, pos_emb, W1, b1, w2, b2=None, **_ignored):
    # b2 shifts every logit equally -> cancels in softmax; unused.
    HP = choose_hp(user_history_mask)
    nc = _get_nc(HP)
    in_maps = make_in_maps(history_repr, candidate_repr, user_history_mask,
                           user_history_position, pos_emb, W1, b1, w2, HP=HP)
    res = bass_utils.run_bass_kernel_spmd(nc, in_maps, list(range(NCORES)))
    ur = np.concatenate([res.results[c]["ur_out"] for c in range(NCORES)], 0)
    cand = np.concatenate([res.results[c]["cand_out"] for c in range(NCORES)], 0)
    return ur, cand


# revision 19
# speedup vs baseline: 1.2667x; 1.0175x over previous
"""Trainium2 Bass kernel for the news-attention module (v5).

Math (exact, unchanged):
  relu(hc + hh) = max(hc, -hh) + hh
  logits a[n,h] = w2 . max(hc[n], -hh[h]) + sh[h],  sh = w2 . hh
  softmax(a + maskbias) = exp(w2.max-part) * m[h] / sum,
      m[h] = mask[h] * exp(sh[h])
  ur[n] = (1/ssum) sum_h exnorm[n,h] * (m[h] * histf[h])

v5 = v4 pipeline + masked-history compaction: rows with mask=0
contribute exactly 0 (m[h]=0), so the host gathers only kept rows and
pads each batch to HP (= even ceil of the max kept count, <= 50); pad
rows get mask 0 and behave identically to masked rows. The entire
history pipeline (hh GEMM, max pass, matvec, attention K) runs at HP
instead of 50. The module is compiled per HP on first use.

Sharding: data-parallel over batch, 8 batches per core on 8 cores.
"""

import sys

for _p in ("/opt/trn_rl_repo",):
    if _p not in sys.path:
        sys.path.insert(0, _p)

import numpy as np
import ml_dtypes

import concourse.bass as bass
import concourse.bacc as bacc
import concourse.tile as tile
from concourse import mybir
from concourse import bass_utils
from concourse.masks import make_identity

DT = mybir.dt.float32
BF = mybir.dt.bfloat16
I32 = mybir.dt.int32
AF = mybir.ActivationFunctionType
ALU = mybir.AluOpType

NCORES = 8
B = 64
BC = B // NCORES  # 8 batches per core
H = 50
N = 50
D = 400
P = 100
A = 200
F = D + P       # 500
J = 52
A0 = 128        # a-chunk split: [0:128), [128:200)
A1 = A - A0     # 72
NT = 7          # matvec n-tiles per batch (8 n-locals each)
NG = 4          # batch groups (2 batches each)
GB = BC // NG   # batches per group = 2
NSG = NT * GB   # 14 psum slots per group


def _bc_(v, pos, n):
    """Insert a zero-stride (broadcast) dim of length n at position pos."""
    ap = [list(x) for x in v.ap]
    ap.insert(pos, [0, n])
    return bass.AP(tensor=v.tensor, offset=v.offset, ap=ap)


def _ap(v, offset_delta, ap_list):
    return bass.AP(tensor=v.tensor, offset=v.offset + offset_delta, ap=ap_list)


def _body(nc, hist_in, cand_in, maskT_in, pos_in, pos_emb16, pos_emb32,
          posT16, w1t, b1_16, w2_16, ur_out, cand_out, tc, HP):
    import contextlib

    HW = 32 if HP <= 32 else 64          # psum h-padding per n-local
    NWIN = (8 * HW) // 128               # transpose windows per slot row
    PARS = 128 // HW                     # n-locals per window

    ctx = contextlib.ExitStack()
    with ctx:
        consts = ctx.enter_context(tc.tile_pool(name="consts", bufs=1))
        ps = ctx.enter_context(tc.tile_pool(name="ps", bufs=4, space="PSUM"))
        psmv = ctx.enter_context(tc.tile_pool(name="psmv", bufs=2, space="PSUM"))
        psat = ctx.enter_context(tc.tile_pool(name="psat", bufs=2, space="PSUM"))
        hidp = ctx.enter_context(tc.tile_pool(name="hid", bufs=4))
        exp_ = ctx.enter_context(tc.tile_pool(name="exp", bufs=2))

        # ---------------- constants ----------------
        ident16 = consts.tile([128, 128], BF, bufs=2)
        make_identity(nc, ident16)
        ident32 = consts.tile([128, 128], DT)
        make_identity(nc, ident32)

        # W1T[f, a] in 10 f-chunks of 100 (host provides W1 transposed, bf16)
        w1T = consts.tile([100, 10, A], BF)
        nc.sync.dma_start(out=w1T,
                          in_=w1t.ap().rearrange("(k p) a -> p k a", p=100))

        pos_emb_s = consts.tile([J, P], BF)
        nc.gpsimd.dma_start(out=pos_emb_s, in_=pos_emb16.ap())
        posT = consts.tile([P, J], BF)
        nc.gpsimd.dma_start(out=posT, in_=posT16.ap())
        maskT_s = consts.tile([HP, BC], DT)
        nc.gpsimd.dma_start(out=maskT_s, in_=maskT_in.ap())
        b1row = consts.tile([1, A], BF)
        nc.gpsimd.dma_start(out=b1row, in_=_ap(b1_16.ap(), 0, [[0, 1], [1, A]]))
        one11 = consts.tile([1, 1], BF)
        nc.vector.memset(one11, 1.0)

        # w2 chunks as columns + shifted zero-window tensors for slot matvec
        w2colA = consts.tile([A0, 1], BF)
        nc.gpsimd.dma_start(out=w2colA, in_=_ap(w2_16.ap(), 0, [[1, A0], [1, 1]]))
        w2colB = consts.tile([A1, 1], BF)
        nc.gpsimd.dma_start(out=w2colB, in_=_ap(w2_16.ap(), A0, [[1, A1], [1, 1]]))
        ZA = consts.tile([A0, 2 * NSG - 1], BF, bufs=2)
        ZB = consts.tile([A1, 2 * NSG - 1], BF, bufs=2)
        nc.vector.memset(ZA, 0.0)
        nc.vector.memset(ZB, 0.0)
        nc.vector.tensor_copy(out=ZA[:, NSG - 1:NSG], in_=w2colA)
        nc.vector.tensor_copy(out=ZB[:, NSG - 1:NSG], in_=w2colB)

        # E[j, a] = pos_emb @ Wh2.T  (Wh2 = W1[:, 900:1000] rows of w1t)
        E_s = consts.tile([J, A], BF)
        psE = ps.tile([J, A], DT, tag="ps")
        nc.tensor.matmul(psE, lhsT=posT[:, :], rhs=w1T[:, 9, :],
                         start=True, stop=True)
        nc.scalar.copy(out=E_s, in_=psE)

        # c0[a] = Wc2 @ pos_emb[1] + b1 as per-partition bias columns
        c0colA = consts.tile([A0, 1], DT)
        c0colB = consts.tile([A1, 1], DT)
        for c0c, asl in ((c0colA, slice(0, A0)), (c0colB, slice(A0, A))):
            psc = ps.tile([c0c.shape[0], 1], DT, tag="ps")
            nc.tensor.matmul(psc, lhsT=w1T[:, 4, asl], rhs=posT[:, 1:2],
                             start=True, stop=False)
            nc.tensor.matmul(psc, lhsT=b1row[:, asl], rhs=one11[:, :],
                             start=False, stop=True)
            nc.scalar.copy(out=c0c, in_=psc)

        # one-hot of positions, transposed: onehot[j, b*HP+h] = (pos[b,h]==j)
        pos52 = consts.tile([J, BC * HP], I32)
        nc.gpsimd.dma_start(out=pos52, in_=_bc_(pos_in.ap(), 0, J))
        iot = consts.tile([J, BC * HP], I32)
        nc.gpsimd.iota(iot, pattern=[[0, BC * HP]], base=0, channel_multiplier=1)
        onehot = consts.tile([J, BC * HP], BF)
        nc.vector.tensor_tensor(out=onehot, in0=iot, in1=pos52, op=ALU.is_equal)

        # ---------------- full-input loads --------------------------------
        cand_all = consts.tile([100, 4, D], DT)   # [2x50 rows, batch-pair, feat]
        hist_all = consts.tile([2 * HP, 4, D], DT)
        for hf in range(2):
            src_c = _ap(cand_in.ap(), hf * N * D,
                        [[D, 50], [2 * N * D, 4], [1, D]])
            src_h = _ap(hist_in.ap(), hf * HP * D,
                        [[D, HP], [2 * HP * D, 4], [1, D]])
            nc.sync.dma_start(out=cand_all[hf * 50:(hf + 1) * 50, :, :],
                              in_=src_c)
            nc.sync.dma_start(out=hist_all[hf * HP:(hf + 1) * HP, :, :],
                              in_=src_h)

        # candidate passthrough: two strided DMAs + pos_emb[1] broadcast
        for hf in range(2):
            for p in range(4):
                dst = _ap(cand_out.ap(), hf * N * F + p * 2 * N * F,
                          [[F, 50], [1, D]])
                nc.sync.dma_start(
                    out=dst, in_=cand_all[hf * 50:(hf + 1) * 50, p, :])
        nc.gpsimd.dma_start(
            out=cand_out.ap()[:, :, D:F],
            in_=_bc_(_bc_(pos_emb32.ap()[1:2, :], 0, N), 0, BC))

        # hist rows re-staged [h, b, f] for the attention V matrix
        staging = consts.tile([HP, BC, D], DT)
        src_hf = _ap(hist_in.ap(), 0, [[D, HP], [HP * D, BC], [1, D]])
        nc.sync.dma_start(out=staging, in_=src_hf)

        # ---------------- persistent SBUF tensors -------------------------
        candT = consts.tile([100, 4, BC * N], BF)  # [feat-chunk, k, (b,n)]
        histT = consts.tile([100, 4, BC * HP], BF)
        hcT2A = consts.tile([A0, BC * N, 2], BF)
        hcT2B = consts.tile([A1, BC * N, 2], BF)
        neghhA = consts.tile([A0, BC * HP], BF)
        neghhB = consts.tile([A1, BC * HP], BF)
        mw = consts.tile([HP, BC], DT)
        histf16 = consts.tile([HP, BC, F + 1], BF)
        alphaT = consts.tile([HP, BC, 64], BF)
        urs = consts.tile([N, BC, F], DT)
        rs_all = consts.tile([N, BC], DT)

        # ---------------- transposes (all groups) + wide GEMMs ------------
        for g in range(4):
            ptc = ps.tile([100, 4, 100], DT, tag="ps")
            pth = ps.tile([100, 4, 2 * HP], DT, tag="ps")
            for k in range(4):
                nc.tensor.transpose(
                    ptc[:, k, :], cand_all[:, g, k * 100:(k + 1) * 100],
                    ident32[:100, :100])
                nc.tensor.transpose(
                    pth[:, k, :], hist_all[:, g, k * 100:(k + 1) * 100],
                    ident32[:2 * HP, :2 * HP])
            nc.scalar.copy(out=candT[:, :, g * 100:(g + 1) * 100], in_=ptc)
            nc.scalar.copy(out=histT[:, :, g * 2 * HP:(g + 1) * 2 * HP],
                           in_=pth)

        for asl, hcT2, neghh, c0c in (
                (slice(0, A0), hcT2A, neghhA, c0colA),
                (slice(A0, A), hcT2B, neghhB, c0colB)):
            m = asl.stop - asl.start
            pg = ps.tile([m, BC * N], DT, tag="ps")
            for k in range(4):
                nc.tensor.matmul(pg, lhsT=w1T[:, k, asl], rhs=candT[:, k, :],
                                 start=(k == 0), stop=(k == 3))
            nc.scalar.activation(out=hcT2, in_=_bc_(pg[:, :], 2, 2),
                                 func=AF.Identity, bias=c0c, scale=1.0)
            ph = ps.tile([m, BC * HP], DT, tag="ps")
            for k in range(4):
                nc.tensor.matmul(ph, lhsT=w1T[:, 5 + k, asl],
                                 rhs=histT[:, k, :],
                                 start=(k == 0), stop=False)
            nc.tensor.matmul(ph, lhsT=E_s[:, asl], rhs=onehot[:, :],
                             start=False, stop=True)
            nc.scalar.activation(out=neghh, in_=ph, func=AF.Copy, scale=-1.0)

        # ---------------- per-group prep: shT + mw + histf ----------------
        def head(g):
            psh = ps.tile([HP, GB], DT, tag="ps")
            for bl in range(GB):
                b = g * GB + bl
                hsl = slice(b * HP, (b + 1) * HP)
                nc.tensor.matmul(psh[:, bl:bl + 1], lhsT=neghhA[:, hsl],
                                 rhs=w2colA, start=True, stop=False)
                nc.tensor.matmul(psh[:, bl:bl + 1], lhsT=neghhB[:, hsl],
                                 rhs=w2colB, start=False, stop=True)
            shE = exp_.tile([HP, GB], DT, tag="she")
            nc.scalar.activation(out=shE, in_=psh, func=AF.Exp, scale=-1.0)
            bsl = slice(g * GB, (g + 1) * GB)
            nc.vector.tensor_tensor(out=mw[:, bsl], in0=shE,
                                    in1=maskT_s[:, bsl], op=ALU.mult)

            # histf (V matrix) with m-scale + m column (ssum source);
            # pos-emb part straight from psum.
            for bl in range(GB):
                b = g * GB + bl
                nc.scalar.activation(out=histf16[:, b, 0:D],
                                     in_=staging[:, b, :],
                                     func=AF.Copy, scale=mw[:, b:b + 1])
                ppg = ps.tile([HP, P], DT, tag="ps")
                nc.tensor.matmul(ppg, lhsT=onehot[:, b * HP:(b + 1) * HP],
                                 rhs=pos_emb_s[:, :], start=True, stop=True)
                nc.scalar.activation(out=histf16[:, b, D:F], in_=ppg,
                                     func=AF.Copy, scale=mw[:, b:b + 1])
                nc.scalar.copy(out=histf16[:, b, F:F + 1], in_=mw[:, b:b + 1])

        # ---------------- per-group matvec (max pass + slot matmuls) ------
        def matvec(g):
            pmv = psmv.tile([NSG, 8, HW], DT, tag="pmv")
            first = True
            for bl in range(GB):
                b = g * GB + bl
                nsl = slice(b * N, (b + 1) * N)
                hids = []
                for hcT2, neghh, aw in ((hcT2A, neghhA, A0),
                                        (hcT2B, neghhB, A1)):
                    hid = hidp.tile([aw, N * HP], BF, tag=f"hid{aw}")
                    v = hcT2[:, nsl, :]
                    hcb = _bc_(v, 2, HP // 2)           # [aw, 50, HP/2, 2]
                    w = neghh[:, b * HP:(b + 1) * HP]
                    hhb = _ap(w, 0, [list(w.ap[0]), [0, N], [2, HP // 2],
                                     [1, 2]])
                    hidv = _ap(hid, 0, [list(hid.ap[0]), [HP, N],
                                        [2, HP // 2], [1, 2]])
                    nc.vector.tensor_tensor(out=hidv, in0=hcb, in1=hhb,
                                            op=ALU.max)
                    hids.append(hid)
                for Z, hv in ((ZA, hids[0]), (ZB, hids[1])):
                    for t in range(NT):
                        s = t * GB + bl
                        nn = 8 if t < 6 else 2   # n-cols in this tile
                        rsl = slice(t * 8 * HP, (t * 8 + nn) * HP)
                        dst = _ap(pmv, 0, [list(pmv.ap[0]), [HW, nn],
                                           [1, HP]])
                        last = (bl == GB - 1 and t == NT - 1 and Z is ZB)
                        nc.tensor.matmul(
                            dst, lhsT=Z[:, NSG - 1 - s:2 * NSG - 1 - s],
                            rhs=hv[:, rsl], start=first, stop=last)
                        first = False
            return pmv

        # ---------------- per-group tail: exp -> alpha -> attention -------
        def tail(g, pmv):
            ex = exp_.tile([NSG, 8, HW], BF, tag="ex")
            nc.scalar.activation(out=ex[:, :, 0:HP], in_=pmv[:, :, 0:HP],
                                 func=AF.Exp)
            ptr = ps.tile([128, NWIN, NSG], BF, tag="ps")
            for w in range(NWIN):
                nc.tensor.transpose(
                    ptr[:, w, :],
                    _ap(ex, w * 128, [list(ex.ap[0]), [1, 128]]),
                    ident16[:NSG, :NSG])
            exT = exp_.tile([128, NWIN, NSG], BF, tag="exT")
            nc.scalar.copy(out=exT, in_=ptr)
            # alpha assembly on Pool: alphaT[h, b, n], n = 8t + PARS*w + par
            # (one copy per par covers all windows w)
            for par in range(PARS):
                src = exT[par * HW:par * HW + HP, :, :]
                nc.gpsimd.tensor_copy(
                    out=_ap(alphaT, g * GB * 64 + par,
                            [list(alphaT.ap[0]), [PARS, NWIN], [8, NT],
                             [64, GB]]),
                    in_=_ap(src, 0, [list(src.ap[0]), [NSG, NWIN], [GB, NT],
                                     [1, GB]]))
            # attention: ur = alpha @ histf / ssum
            for bl in range(GB):
                b = g * GB + bl
                pur = psat.tile([N, F + 1], DT, tag="pur")
                nc.tensor.matmul(pur, lhsT=alphaT[:, b, 0:N],
                                 rhs=histf16[:, b, :], start=True, stop=True)
                nc.vector.reciprocal(rs_all[:, b:b + 1], pur[:, F:F + 1])
                nc.scalar.activation(out=urs[:, b, :], in_=pur[:, 0:F],
                                     func=AF.Copy, scale=rs_all[:, b:b + 1])
            for bl in range(GB):
                b = g * GB + bl
                dst_ur = _ap(ur_out.ap(), b * N * F, [[F, N], [1, F]])
                nc.sync.dma_start(out=dst_ur, in_=urs[:, b, :])

        # ---------------- software pipeline ------------------------------
        head(0)
        head(1)
        pmv0 = matvec(0)
        head(2)
        pmv1 = matvec(1)
        tail(0, pmv0)
        head(3)
        pmv2 = matvec(2)
        tail(1, pmv1)
        pmv3 = matvec(3)
        tail(2, pmv2)
        tail(3, pmv3)


def build(debug=False, reps=1, loop=1, HP=H):
    nc = bacc.Bacc("TRN2", target_bir_lowering=False, debug=debug)
    hist_in = nc.dram_tensor("hist_in", [BC, HP, D], DT, kind="ExternalInput")
    cand_in = nc.dram_tensor("cand_in", [BC, N, D], DT, kind="ExternalInput")
    maskT_in = nc.dram_tensor("maskT_in", [HP, BC], DT, kind="ExternalInput")
    pos_in = nc.dram_tensor("pos_in", [BC, HP], I32, kind="ExternalInput")
    pos_emb16 = nc.dram_tensor("pos_emb16", [J, P], BF, kind="ExternalInput")
    pos_emb32 = nc.dram_tensor("pos_emb32", [J, P], DT, kind="ExternalInput")
    posT16 = nc.dram_tensor("posT16", [P, J], BF, kind="ExternalInput")
    w1t = nc.dram_tensor("w1t", [2 * F, A], BF, kind="ExternalInput")
    b1_16 = nc.dram_tensor("b1_16", [A], BF, kind="ExternalInput")
    w2_16 = nc.dram_tensor("w2_16", [A], BF, kind="ExternalInput")
    ur_out = nc.dram_tensor("ur_out", [BC, N, F], DT, kind="ExternalOutput")
    cand_out = nc.dram_tensor("cand_out", [BC, N, F], DT, kind="ExternalOutput")

    with tile.TileContext(nc) as tc:
        if loop > 1:
            with tc.For_i(0, loop):
                _body(nc, hist_in, cand_in, maskT_in, pos_in, pos_emb16,
                      pos_emb32, posT16, w1t, b1_16, w2_16, ur_out,
                      cand_out, tc, HP)
        else:
            for _ in range(reps):
                _body(nc, hist_in, cand_in, maskT_in, pos_in, pos_emb16,
                      pos_emb32, posT16, w1t, b1_16, w2_16, ur_out,
                      cand_out, tc, HP)
    nc.compile()
    return nc


_NCS = {}


def _get_nc(HP):
    if HP not in _NCS:
        _NCS[HP] = build(debug=False, HP=HP)
    return _NCS[HP]


def _bf(x):
    return np.ascontiguousarray(np.asarray(x, np.float32).astype(
        ml_dtypes.bfloat16))


def choose_hp(user_history_mask):
    k = int(np.asarray(user_history_mask).astype(bool).sum(axis=1).max())
    hp = max(2, k + (k % 2))
    return min(hp, H)


def compact(history_repr, user_history_mask, user_history_position, HP):
    """Gather kept (mask=1) history rows, pad to HP; pad rows get mask 0."""
    hist = np.asarray(history_repr, np.float32)
    mask = np.asarray(user_history_mask).astype(bool)
    pos = np.asarray(user_history_position).astype(np.int32)
    Bn = hist.shape[0]
    hist_g = np.zeros((Bn, HP, hist.shape[2]), np.float32)
    pos_g = np.zeros((Bn, HP), np.int32)
    mask_g = np.zeros((Bn, HP), np.float32)
    for b in range(Bn):
        idx = np.flatnonzero(mask[b])[:HP]
        k = len(idx)
        hist_g[b, :k] = hist[b, idx]
        pos_g[b, :k] = pos[b, idx]
        mask_g[b, :k] = 1.0
    return hist_g, pos_g, mask_g


def make_in_maps(history_repr, candidate_repr, user_history_mask,
                 user_history_position, pos_emb, W1, b1, w2, HP=None):
    if HP is None:
        HP = choose_hp(user_history_mask)
    hist_g, pos_g, mask_g = compact(history_repr, user_history_mask,
                                    user_history_position, HP)
    cand = np.ascontiguousarray(np.asarray(candidate_repr, np.float32))
    pe32 = np.ascontiguousarray(np.asarray(pos_emb, np.float32))
    pe16 = _bf(pe32)
    peT16 = _bf(pe32.T)
    w1t16 = _bf(np.asarray(W1, np.float32).T)
    b1_16 = _bf(b1)
    w2_16 = _bf(w2)
    in_maps = []
    for c in range(NCORES):
        sl = slice(c * BC, (c + 1) * BC)
        in_maps.append({
            "hist_in": np.ascontiguousarray(hist_g[sl]),
            "cand_in": cand[sl],
            "maskT_in": np.ascontiguousarray(mask_g[sl].T),
            "pos_in": np.ascontiguousarray(pos_g[sl]),
            "pos_emb16": pe16, "pos_emb32": pe32, "posT16": peT16,
            "w1t": w1t16, "b1_16": b1_16, "w2_16": w2_16,
        })
    return in_maps


def kernel(history_repr, candidate_repr, user_history_mask,
           user_history_position, pos_emb, W1, b1, w2, b2=None, **_ignored):
    # b2 shifts every logit equally -> cancels in softmax; unused.
    HP = choose_hp(user_history_mask)
    nc = _get_nc(HP)
    in_maps = make_in_maps(history_repr, candidate_repr, user_history_mask,
                           user_history_position, pos_emb, W1, b1, w2, HP=HP)
    res = bass_utils.run_bass_kernel_spmd(nc, in_maps, list(range(NCORES)))
    ur = np.concatenate([res.results[c]["ur_out"] for c in range(NCORES)], 0)
    cand = np.concatenate([res.results[c]["cand_out"] for c in range(NCORES)], 0)
    return ur, cand
